# revision 1
# baseline (speedup 1.0000x reference)
"""MoE routing gate kernel for Trainium2 (8 NeuronCores, data-parallel).

Computes, for x[32768, 2048], weight[64, 2048], bias[64]:
    logits = x @ weight.T
    probs  = softmax(logits, axis=-1)
    idx    = top_k(probs + bias, 6).indices
    w      = take_along_axis(probs, idx)
returning (w float32 [32768, 6], idx int32 [32768, 6]).

Sharding: tokens split 4096/core across 8 cores; weight/bias replicated.
DMA: hi and lo chunks ride the sync ring interleaved in consumption order;
the last super-group uses dedicated SBUF tiles so its fetch is never gated
on the PE freeing earlier buffers.

Per-core pipeline (memory-bound; HBM floor ~60us for the 25MB shard):
  - x is streamed at 3 bytes/element: fp16 hi + fp8e4m3 lo with
    lo = (x - fp16(x)) * 2048 (quantized into e4m3's normal range).
    Three matmul passes accumulate fp32 logits in PSUM:
      hi @ w_hi(fp16) + hi @ w_lo(fp16) + lo8 @ w3(fp16, = w_hi/2048)
    giving |logit err| ~2.5e-5 (equivalent to the fp32 reference for
    top-k stability) while cutting HBM read traffic 25% vs fp32.
  - Matmul pairs are column-tiled: group g=0 lands in PE columns 0-63
    (PSUM partitions 0-63), g=1 in columns 64-127, so two N=512 matmuls
    stream concurrently through disjoint column groups of the array.
  - logits^T -> ACT copy to SBUF -> 8 PE transposes (identity matmul)
    into two PSUM tiles per super-group (transposes reading partition
    bases 0 and 64 must not share a PSUM bank - that hangs the HW).
  - Softmax without max-subtraction (|logits| < ~7, exp is safe in
    fp32): per-j ACT exp emits the row sum via accum_out; DVE ranks
    q = exp + sum*bias (same ordering as probs + bias) with
    Max8/MaxIndex8 and only the top-6 INDICES leave the device.
  - The exp values themselves are DMA'd out raw (1MB/core, ~2% extra
    HBM traffic); the host computes probs = exp/sum and gathers the
    top-6 weights during unpacking. This removes the expensive
    on-device one-hot gather (was ~40us of DVE time) entirely.
"""

import numpy as np
import ml_dtypes

import concourse.bacc as bacc
import concourse.bass as bass
import concourse.mybir as mybir
import concourse.tile as tile
from concourse.bass_utils import run_bass_kernel_spmd

F32 = mybir.dt.float32
F16 = mybir.dt.float16
F8E4 = mybir.dt.float8e4
I32 = mybir.dt.int32
U32 = mybir.dt.uint32
AX = mybir.AxisListType
OP = mybir.AluOpType
EXP = mybir.ActivationFunctionType.Exp

TOKENS, DIM, E, TOPK, NCORES = 32768, 2048, 64, 6, 8
KC = DIM // 128  # contraction chunks of 128
KQ = 4           # k-chunks per DMA


def build_nc(tpc, sg_t=1024):
    """Build the per-core Bass program for a tpc-token shard."""
    grp = sg_t // 2         # tokens per matmul (N), two col-tiled groups per sg
    assert grp == 512
    nsg = tpc // sg_t
    nj = sg_t // 128        # 128-token tiles per super-group
    cols = nj * TOPK        # staging cols per sg

    nc = bacc.Bacc("TRN2", target_bir_lowering=False, debug=False)

    xhi = nc.dram_tensor(
        "xhi", [nsg, KC // KQ, 128, KQ, sg_t], F16, kind="ExternalInput"
    )
    xlo = nc.dram_tensor(
        "xlo", [nsg, KC // KQ, 128, KQ, sg_t], F8E4, kind="ExternalInput"
    )
    wt_hi = nc.dram_tensor("wt_hi", [128, KC, E], F16, kind="ExternalInput")
    wt_lo = nc.dram_tensor("wt_lo", [128, KC, E], F16, kind="ExternalInput")
    wt_3 = nc.dram_tensor("wt_3", [128, KC, E], F16, kind="ExternalInput")
    bias_b = nc.dram_tensor("bias_b", [128, E], F32, kind="ExternalInput")
    ident2 = nc.dram_tensor("ident2", [128, 64], F32, kind="ExternalInput")
    ex_out = nc.dram_tensor("ex_out", [nsg, 128, nj, E], F32, kind="ExternalOutput")
    i_out = nc.dram_tensor("i_out", [nsg, 128, cols], I32, kind="ExternalOutput")

    with tile.TileContext(nc) as tc:
        with (
            tc.tile_pool(name="consts", bufs=1) as cpool,
            tc.tile_pool(name="xh", bufs=8) as xhp,
            tc.tile_pool(name="xl", bufs=8) as xlp,
            tc.tile_pool(name="xh3", bufs=4) as xh3p,
            tc.tile_pool(name="xl3", bufs=4) as xl3p,
            tc.tile_pool(name="lt", bufs=3) as ltp,
            tc.tile_pool(name="ex", bufs=3) as exp_,
            tc.tile_pool(name="wk", bufs=2) as wkp,
            tc.tile_pool(name="small", bufs=3) as smp,
            tc.tile_pool(name="stage", bufs=3) as stp,
            tc.tile_pool(name="acc", bufs=3, space="PSUM") as accp,
            tc.tile_pool(name="tr", bufs=2, space="PSUM") as trp,
        ):
            cwh = cpool.tile([128, KC, E], F16)
            nc.scalar.dma_start(cwh, wt_hi[:])
            cwl = cpool.tile([128, KC, E], F16)
            nc.scalar.dma_start(cwl, wt_lo[:])
            cw3 = cpool.tile([128, KC, E], F16)
            nc.scalar.dma_start(cw3, wt_3[:])
            cbias = cpool.tile([128, E], F32)
            nc.scalar.dma_start(cbias, bias_b[:])
            cident = cpool.tile([128, 64], F32)
            nc.scalar.dma_start(cident, ident2[:])

            def finish_sg(sg, acc):
                """Copy/transpose/softmax/rank/out for a finished super-group.

                Deferred one sg behind the matmul issue so the Tensor queue
                always has the next sg's matmuls ahead of these transposes
                (which wait on the ACT copy) - avoids a cross-engine convoy.
                """
                lt = ltp.tile([128, grp], F32)
                nc.scalar.copy(lt[0:64], acc[0:64])
                nc.scalar.copy(lt[64:128], acc[64:128])

                # 8 transposes into two PSUM tiles [128 tok, nj/2, 64 exp].
                # NB: transposes reading partition bases 0 and 64 must land in
                # different PSUM banks - mixing them in one bank hangs the HW.
                tpsA = trp.tile([128, nj // 2, E], F32, tag="tpsA")
                tpsB = trp.tile([128, nj // 2, E], F32, tag="tpsB")
                for j in range(nj):
                    base = 64 * (j // 4)
                    tps = tpsA if j < 4 else tpsB
                    nc.tensor.transpose(
                        tps[:, j % 4],
                        lt[base:base + 64, (j % 4) * 128:(j % 4 + 1) * 128],
                        cident[base:base + 64, :],
                    )

                # per-j ACT exp (accum_out = row sum); q = exp + sum*bias on
                # DVE ranks identically to probs + bias
                ex = exp_.tile([128, nj, E], F32, tag="ex")
                ssum = smp.tile([128, nj], F32, tag="ssum")
                q = wkp.tile([128, nj, E], F32, tag="q")
                mx = smp.tile([128, nj, 8], F32, tag="mx")
                mi = smp.tile([128, nj, 8], U32, tag="mi")
                for j in range(nj):
                    tps = (tpsA if j < 4 else tpsB)[:, j % 4]
                    nc.scalar.activation(
                        ex[:, j], tps, EXP, accum_out=ssum[:, j:j + 1]
                    )
                    nc.vector.scalar_tensor_tensor(
                        q[:, j], cbias, ssum[:, j:j + 1], ex[:, j],
                        OP.mult, OP.add,
                    )
                    nc.vector.max(mx[:, j], q[:, j])
                    nc.vector.max_index(mi[:, j], mx[:, j], q[:, j])

                si = stp.tile([128, nj, TOPK], I32, tag="si")
                nc.vector.tensor_copy(si, mi[:, :, 0:TOPK])

                nc.gpsimd.dma_start(ex_out[sg], ex)
                nc.gpsimd.dma_start(i_out[sg], si.rearrange("p a b -> p (a b)"))

            pending = None  # (sg, acc) awaiting finish
            for sg in range(nsg):
                # x super-group: KQ-chunk DMAs, hi+lo interleaved on sync.
                # The last sg gets DEDICATED tiles: with shared pools its
                # DMA issue is gated on the PE freeing sg0's buffers, which
                # stalls the end of the stream whenever compute runs slow.
                hp = xh3p if sg == nsg - 1 else xhp
                lp = xl3p if sg == nsg - 1 else xlp
                xh, xl = [], []
                for kq in range(KC // KQ):
                    th = hp.tile([128, KQ, sg_t], F16, tag="xh")
                    nc.sync.dma_start(th, xhi[sg, kq])
                    xh.append(th)
                    tl = lp.tile([128, KQ, sg_t], F8E4, tag="xl")
                    # lo rides the sync ring right behind its hi chunk: queue
                    # order then matches consumption order exactly (on the
                    # scalar ring, lo issue sits behind earlier sgs' exp
                    # chains and lands after future sgs' hi bytes)
                    nc.sync.dma_start(tl, xlo[sg, kq])
                    xl.append(tl)

                # 96 matmuls: col-tiled pairs (g=0 -> cols 0-63, g=1 -> 64-127).
                # The previous sg's finish-phase instructions are issued after
                # this sg's first k-chunk so the in-order Tensor queue runs its
                # transposes inside a DMA-wait bubble instead of stalling the
                # matmul stream (and only the last sg's finish is in the tail).
                acc = accp.tile([128, grp], F32)
                for k in range(KC):
                    hi_k = xh[k // KQ][:, k % KQ]   # [128, sg_t] fp16
                    lo_k = xl[k // KQ][:, k % KQ]   # [128, sg_t] fp8
                    for p in range(3):
                        w = (cwh, cwl, cw3)[p][:, k, :]
                        xs = (hi_k, hi_k, lo_k)[p]
                        first, last = (k == 0 and p == 0), (k == KC - 1 and p == 2)
                        nc.tensor.matmul(
                            acc[0:64], w, xs[:, 0:grp],
                            start=first, stop=last, tile_position=(0, 0),
                        )
                        nc.tensor.matmul(
                            acc[64:128], w, xs[:, grp:sg_t],
                            start=first, stop=last, tile_position=(0, 64),
                            skip_group_check=True,
                        )
                    if k == KQ - 1 and pending is not None:
                        finish_sg(*pending)
                        pending = None
                pending = (sg, acc)
            finish_sg(*pending)
    return nc


_CACHE = {}


def _get_compiled(tpc):
    if tpc not in _CACHE:
        nc = build_nc(tpc)
        nc.compile()
        _CACHE[tpc] = nc
    return _CACHE[tpc]


def _prep_shared(weight, bias):
    f16 = np.float16
    w = np.asarray(weight, np.float32)
    w_hi = w.astype(f16)
    w_lo = (w - w_hi.astype(np.float32)).astype(f16)
    w_3 = (w_hi.astype(np.float32) * (1.0 / 2048.0)).astype(f16)

    def wtile(a):  # [E, DIM] -> [128, KC, E]
        return np.ascontiguousarray(
            np.ascontiguousarray(a.T).reshape(KC, 128, E).transpose(1, 0, 2)
        )

    return {
        "wt_hi": wtile(w_hi),
        "wt_lo": wtile(w_lo),
        "wt_3": wtile(w_3),
        "bias_b": np.ascontiguousarray(
            np.broadcast_to(np.asarray(bias, np.float32), (128, E))
        ),
        "ident2": np.ascontiguousarray(
            np.tile(np.eye(64, dtype=np.float32), (2, 1))
        ),
    }


def prep_core_inputs(x, weight, bias, ncores=NCORES, sg_t=1024):
    f16 = np.float16
    e4 = ml_dtypes.float8_e4m3fn
    shared = _prep_shared(weight, bias)
    x = np.asarray(x, np.float32)
    tpc = x.shape[0] // ncores
    nsg = tpc // sg_t
    # whole-tensor transpose + casts once (not per core)
    xT = np.ascontiguousarray(x.T)           # [DIM, TOKENS]
    xhT = xT.astype(f16)
    xlT = ((xT - xhT.astype(np.float32)) * 2048.0).astype(e4)
    del xT
    in_maps = []
    for c in range(ncores):
        sl = slice(c * tpc, (c + 1) * tpc)
        # pack to [nsg, KC//KQ, 128, KQ, sg_t]: per (sg, kq, partition) the
        # [KQ, sg_t] block is one contiguous run in DRAM (8KB hi / 4KB lo)
        xh6 = xhT[:, sl].reshape(KC // KQ, KQ, 128, nsg, sg_t)
        xl6 = xlT[:, sl].reshape(KC // KQ, KQ, 128, nsg, sg_t)
        in_maps.append({
            "xhi": np.ascontiguousarray(xh6.transpose(3, 0, 2, 1, 4)),
            "xlo": np.ascontiguousarray(xl6.transpose(3, 0, 2, 1, 4)),
            **shared,
        })
    return in_maps


def unpack_outputs(res_list, tpc):
    ws, idxs = [], []
    for r in res_list:
        ev = np.asarray(r["ex_out"])  # [nsg, 128, nj, E]
        iv = np.asarray(r["i_out"])   # [nsg, 128, cols]
        nsg = ev.shape[0]
        nj = ev.shape[2]
        # token t = sg*sg_t + 128*j + p
        ev = ev.transpose(0, 2, 1, 3).reshape(tpc, E)
        iv = iv.reshape(nsg, 128, nj, TOPK).transpose(0, 2, 1, 3).reshape(tpc, TOPK)
        probs = ev / ev.sum(axis=-1, keepdims=True)
        wv = np.take_along_axis(probs, iv, axis=-1)
        ws.append(wv)
        idxs.append(iv)
    return (
        np.ascontiguousarray(np.concatenate(ws)).astype(np.float32),
        np.ascontiguousarray(np.concatenate(idxs)).astype(np.int32),
    )


def run(x, weight, bias, trace=False, **kwargs):
    x = np.asarray(x, np.float32)
    tpc = x.shape[0] // NCORES
    nc = _get_compiled(tpc)
    in_maps = prep_core_inputs(x, weight, bias)
    res = run_bass_kernel_spmd(nc, in_maps, list(range(NCORES)), trace=trace, **kwargs)
    w, i = unpack_outputs(res.results, tpc)
    return w, i, res


def kernel(x, weight, bias):
    w, i, _ = run(x, weight, bias, trace=False)
    return w, i

